# revision 4
# baseline (speedup 1.0000x reference)
"""GAT attention head (nn_AttHead_11330123727477) on 8 Trainium2 NeuronCores.

Reference computation:
    h = input @ W;  e_ij = leakyrelu(f_src_i + f_dst_j, 0.2)
    h' = elu(softmax_j(where(adj, e, -inf)) @ h)

Exact algebraic restructuring (as the previous kernel): with
u'_i = exp(-0.8 f_src_i), v_j = exp(0.8 f_dst_j), q_j = exp(0.2 f_dst_j),
after dividing row i of the softmax numerator by exp(0.8 f_src_i):
    att_ij ∝ A_ij * q_j * max(u'_i, v_j)
    h'_i = elu( (Σ_j att q_j h_j ...) / denominator )

This version ("single-plane scheme") ships ONE encoded score plane per core,
Y[j,i] = where(A^T, u'_i, -B)  (bf16, B=240), and decomposes
    A*max(u',v) = A*v + A*relu(u'-v):
  - relu part: ONE VectorE tensor_scalar per 128-j chunk (4x mode):
        P2 = max(Y - v_j, 0)            (A=0 cells give relu(-B-v)=0 exactly)
        psN += [q h | q]^T @ P2
  - A*v part is recovered ALGEBRAICALLY from the same plane on the PE:
        psM += [q v h | q v]^T @ Y
        Σ_{j∈A} qvh = (psM + B*C) * 1/(u'_i + B)    (exact; C = Σ_j qvh)
    The B*C offset is folded into psM's PSUM group via a K=1 fp32 matmul at
    start; 1/(u'+B) is a host-computed fp32 row, broadcast via ones-matmul.
Epilogue: R = psN + (psM)*rcp_bc; h' = R[:64]/R[64]; elu; store.

vs the previous kernel this cuts VectorE O(N^2) work from ~43us to ~21us
(one 4x-mode tensor_scalar per chunk instead of TT+TS mixes) and ScalarE
O(N^2) work to zero; PE carries two bf16 passes over the plane (~55us).
(fp8 DoubleRow would halve the P2 pass, but its 64-wide stationary limit
cannot carry the 65th (denominator) functional; walrus also crashes on
M>64 DoubleRow. Measured tradeoff is a wash, so everything stays bf16.)

Sharding: row-parallel over the 8192 output rows, 1024 rows per core,
no cross-core communication.
"""

import numpy as np
import ml_dtypes

N = 8192
IN_F = 128
OUT_F = 64
HT_F = OUT_F + 1  # [q*h | q] carries the denominator column
N_CORES = 8
SLAB = N // N_CORES  # 1024 output rows per core
P = 128
NT = N // P  # 64 j-chunks of 128
NPAIR = NT // 2
HALF = SLAB // 2  # PSUM free-dim limit for fp32 is 512
BIG = 240.0

_bf16 = ml_dtypes.bfloat16
_f8 = ml_dtypes.float8_e4m3  # TRN FP8_EXP4 (max +-240)

_nc_cache = None


def _build_bass():
    import concourse.mybir as mybir
    import concourse.tile as tile
    from concourse import bacc

    bf = mybir.dt.bfloat16
    f8 = mybir.dt.float8e4
    f32 = mybir.dt.float32
    Alu = mybir.AluOpType
    DR = mybir.MatmulPerfMode.DoubleRow

    nc = bacc.Bacc("TRN2", target_bir_lowering=False, debug=False)

    plane = nc.dram_tensor("plane", [N, SLAB], bf, kind="ExternalInput")
    vT = nc.dram_tensor("vT", [P, NT], f32, kind="ExternalInput")
    qh8 = nc.dram_tensor("qh8", [P, NT * HT_F], bf, kind="ExternalInput")
    qvh = nc.dram_tensor("qvh", [P, NT * HT_F], bf, kind="ExternalInput")
    rcp = nc.dram_tensor("rcp", [1, SLAB], f32, kind="ExternalInput")
    bcT = nc.dram_tensor("bcT", [1, HT_F], f32, kind="ExternalInput")
    out = nc.dram_tensor("out", [OUT_F, SLAB], f32, kind="ExternalOutput")

    plane_t = plane.rearrange("(t p) i -> t p i", p=P)

    with tile.TileContext(nc) as tc:
        with (
            tc.tile_pool(name="const", bufs=1) as cpool,
            tc.tile_pool(name="mask", bufs=4) as mpool,
            tc.tile_pool(name="p2", bufs=4) as gpool,
            tc.tile_pool(name="ps", bufs=1, space="PSUM") as pspool,
            tc.tile_pool(name="epi", bufs=1) as epool,
        ):
            vT_sb = cpool.tile([P, NT], f32)
            nc.sync.dma_start(vT_sb[:], vT[:])
            qh8_sb = cpool.tile([P, NT, HT_F], bf)
            nc.scalar.dma_start(qh8_sb[:], qh8.rearrange("p (t f) -> p t f", f=HT_F))
            qvh_sb = cpool.tile([P, NT, HT_F], bf)
            nc.scalar.dma_start(qvh_sb[:], qvh.rearrange("p (t f) -> p t f", f=HT_F))
            rcp_sb = cpool.tile([1, SLAB], f32)
            nc.scalar.dma_start(rcp_sb[:], rcp[:])
            bc_sb = cpool.tile([1, HT_F], f32)
            nc.scalar.dma_start(bc_sb[:], bcT[:])

            ones65 = cpool.tile([1, HT_F], f32)
            nc.vector.memset(ones65[:], 1.0)
            onesrow = cpool.tile([1, HALF], f32)
            nc.vector.memset(onesrow[:], 1.0)

            # Warm the ACT exp table during the main loop (ScalarE is idle).
            warm = cpool.tile([P, 8], f32)
            nc.scalar.activation(
                warm[:], vT_sb[:, 0:8], mybir.ActivationFunctionType.Exp
            )

            # PSUM accumulators (one bank each, 8 banks total)
            psN0 = pspool.tile([HT_F, HALF], f32)
            psN1 = pspool.tile([HT_F, HALF], f32)
            psM0 = pspool.tile([HT_F, HALF], f32)
            psM1 = pspool.tile([HT_F, HALF], f32)
            pb0 = pspool.tile([HT_F, HALF], f32)
            pb1 = pspool.tile([HT_F, HALF], f32)
            pd0 = pspool.tile([OUT_F, HALF], f32)
            pd1 = pspool.tile([OUT_F, HALF], f32)

            # rcp broadcast across the 65 output partitions (off critical path)
            nc.tensor.matmul(pb0[:], ones65[:], rcp_sb[:, 0:HALF])
            nc.tensor.matmul(pb1[:], ones65[:], rcp_sb[:, HALF:SLAB])
            pbS = epool.tile([HT_F, SLAB], f32)
            nc.vector.tensor_copy(out=pbS[:, 0:HALF], in_=pb0[:])
            nc.vector.tensor_copy(out=pbS[:, HALF:SLAB], in_=pb1[:])

            # init psM accumulation with the +B*C offset (K=1 fp32 matmul)
            nc.tensor.matmul(psM0[:], bc_sb[:], onesrow[:], start=True, stop=False)
            nc.tensor.matmul(psM1[:], bc_sb[:], onesrow[:], start=True, stop=False)

            # ---- main loop over j-chunks, grouped for DMA batching ----
            groups = [(0, 2), (2, 2)] + [(4 + 6 * k, 6) for k in range(10)]
            for t0g, grp in groups:
                ybf = mpool.tile([P, 6, SLAB], bf, tag="ybf")
                nc.sync.dma_start(
                    ybf[:, 0:grp, :],
                    plane_t[t0g : t0g + grp].rearrange("t p i -> p t i"),
                )
                p2g = gpool.tile([P, 6, SLAB], bf, tag="p2g")
                for b in range(grp):
                    t = t0g + b
                    # P2 = max(Y - v_j, 0)  -- one 4x-mode tensor_scalar
                    nc.vector.tensor_scalar(
                        p2g[:, b, :],
                        ybf[:, b, :],
                        vT_sb[:, t : t + 1],
                        0.0,
                        Alu.subtract,
                        Alu.max,
                    )
                    # psM += qvh_t^T @ Y_t   (extracts Σ_A qvh via algebra)
                    last = t == NT - 1
                    nc.tensor.matmul(
                        psM0[:], qvh_sb[:, t, :], ybf[:, b, 0:HALF],
                        start=False, stop=last,
                    )
                    nc.tensor.matmul(
                        psM1[:], qvh_sb[:, t, :], ybf[:, b, HALF:SLAB],
                        start=False, stop=last,
                    )
                    # psN += qh_t^T @ P2_t   (relu part + its denominator col)
                    nc.tensor.matmul(
                        psN0[:], qh8_sb[:, t, :], p2g[:, b, 0:HALF],
                        start=(t == 0), stop=last,
                    )
                    nc.tensor.matmul(
                        psN1[:], qh8_sb[:, t, :], p2g[:, b, HALF:SLAB],
                        start=(t == 0), stop=last,
                    )

            # ---- epilogue ----
            # R = psN + psM * rcp_bc   (psM already carries +B*C)
            t1 = epool.tile([HT_F, SLAB], f32)
            nc.vector.tensor_tensor(
                t1[:, 0:HALF], pbS[:, 0:HALF], psM0[:], Alu.mult
            )
            nc.vector.tensor_tensor(
                t1[:, HALF:SLAB], pbS[:, HALF:SLAB], psM1[:], Alu.mult
            )
            R = epool.tile([HT_F, SLAB], f32)
            nc.vector.tensor_tensor(R[:, 0:HALF], t1[:, 0:HALF], psN0[:], Alu.add)
            nc.vector.tensor_tensor(
                R[:, HALF:SLAB], t1[:, HALF:SLAB], psN1[:], Alu.add
            )

            # denominator reciprocal: spread the 1024 values over 128 partitions
            den128 = epool.tile([P, SLAB // P], f32)
            nc.sync.dma_start(den128[:], R[OUT_F : OUT_F + 1, :])
            rcp128 = epool.tile([P, SLAB // P], f32)
            nc.vector.reciprocal(out=rcp128[:], in_=den128[:])
            rcpd = epool.tile([1, SLAB], f32)
            nc.sync.dma_start(rcpd[:], rcp128[:])

            ones64 = cpool.tile([1, OUT_F], f32)
            nc.vector.memset(ones64[:], 1.0)
            nc.tensor.matmul(pd0[:], ones64[:], rcpd[:, 0:HALF])
            nc.tensor.matmul(pd1[:], ones64[:], rcpd[:, HALF:SLAB])

            div = epool.tile([OUT_F, SLAB], f32)
            nc.vector.tensor_tensor(
                div[:, 0:HALF], R[0:OUT_F, 0:HALF], pd0[:], Alu.mult
            )
            nc.vector.tensor_tensor(
                div[:, HALF:SLAB], R[0:OUT_F, HALF:SLAB], pd1[:], Alu.mult
            )

            # elu(x) = relu(x) + min(exp(x) - 1, 0)
            ex = epool.tile([OUT_F, SLAB], f32)
            nc.scalar.activation(ex[:], div[:], mybir.ActivationFunctionType.Exp)
            exm = epool.tile([OUT_F, SLAB], f32)
            nc.vector.tensor_scalar(exm[:], ex[:], 1.0, 0.0, Alu.subtract, Alu.min)
            rl = epool.tile([OUT_F, SLAB], f32)
            nc.vector.tensor_scalar(rl[:], div[:], 0.0, None, Alu.max)
            ov = epool.tile([OUT_F, SLAB], f32)
            nc.vector.tensor_tensor(ov[:], exm[:], rl[:], Alu.add)

            nc.sync.dma_start(out[:], ov[:])

    nc.finalize()
    return nc


def _get_nc():
    global _nc_cache
    if _nc_cache is None:
        _nc_cache = _build_bass()
    return _nc_cache


def prepare_inputs(input, adj, W, a):
    """Host-side O(N*F) precompute + input marshaling (elementwise mask remap
    only). Returns per-core input maps for the SPMD bass kernel."""
    f32 = np.float32
    input = np.asarray(input, dtype=f32)
    W = np.asarray(W, dtype=f32)
    a = np.asarray(a, dtype=f32)
    adj = np.asarray(adj)

    h = input @ W  # [N, 64]
    f_src = h @ a[:OUT_F]
    f_dst = h @ a[OUT_F:]

    u_b = np.exp(-0.8 * f_src).astype(_bf16)  # u' = exp(-0.8 f_src), bf16
    u_f = u_b.astype(f32)
    v = np.exp(0.8 * f_dst).astype(f32)
    q = np.exp(0.2 * f_dst).astype(f32)

    htil = np.empty((N, HT_F), f32)
    htil[:, :OUT_F] = h * q[:, None]
    htil[:, OUT_F] = q
    htil8 = htil.astype(_bf16)
    hv = (htil * v[:, None]).astype(_bf16)

    def dev_layout(x):
        # partition p holds chunk t at columns [t*65, (t+1)*65)
        return np.ascontiguousarray(
            x.reshape(NT, P, HT_F).transpose(1, 0, 2).reshape(P, NT * HT_F)
        )

    qh8_dev = dev_layout(htil8)
    qvh_dev = dev_layout(hv)

    C = hv.astype(np.float64).sum(axis=0)  # consistent with bf16 stationary
    bcT_dev = (BIG * C).astype(f32).reshape(1, HT_F)

    vT_dev = np.ascontiguousarray(v.reshape(NT, P).T)  # [128, 64] f32

    # plane[j, i] = where(adj^T, u'_i, -240) in bf16, built via uint16 bits
    u_bits = u_b.view(np.uint16)
    NEG_BITS = np.uint16(np.float32(-BIG).astype(_bf16).view(np.uint16))
    adjT = np.ascontiguousarray(adj.T != 0)  # [N(j), N(i)]

    in_maps = []
    for c in range(N_CORES):
        sl = slice(c * SLAB, (c + 1) * SLAB)
        plane_c = np.where(adjT[:, sl], u_bits[None, sl], NEG_BITS).view(_bf16)
        rcp_c = (1.0 / (u_f[sl] + BIG)).astype(f32).reshape(1, SLAB)
        in_maps.append(
            {
                "plane": plane_c,
                "vT": vT_dev,
                "qh8": qh8_dev,
                "qvh": qvh_dev,
                "rcp": rcp_c,
                "bcT": bcT_dev,
            }
        )
    return in_maps


def assemble_output(results):
    """results: list of 8 dicts with 'out' [64, 1024] f32 -> [N, 64] f32."""
    hp = np.empty((N, OUT_F), np.float32)
    for c in range(N_CORES):
        hp[c * SLAB : (c + 1) * SLAB] = results[c]["out"].T
    return hp


def kernel(input, adj, W, a):
    import time

    from concourse.bass_utils import run_bass_kernel_spmd

    nc = _get_nc()
    in_maps = prepare_inputs(input, adj, W, a)
    last_err = None
    for attempt in range(3):
        try:
            res = run_bass_kernel_spmd(nc, in_maps, core_ids=list(range(N_CORES)))
            return assemble_output(res.results)
        except Exception as e:  # transient device wedges have been observed
            last_err = e
            time.sleep(5)
    raise last_err


# revision 7
# speedup vs baseline: 1.0367x; 1.0367x over previous
"""GAT attention head (nn_AttHead_11330123727477) on 8 Trainium2 NeuronCores.

Reference computation:
    h = input @ W;  e_ij = leakyrelu(f_src_i + f_dst_j, 0.2)
    h' = elu(softmax_j(where(adj, e, -inf)) @ h)

Exact algebraic restructuring (as the previous kernel): with
u'_i = exp(-0.8 f_src_i), v_j = exp(0.8 f_dst_j), q_j = exp(0.2 f_dst_j),
after dividing row i of the softmax numerator by exp(0.8 f_src_i):
    att_ij ∝ A_ij * q_j * max(u'_i, v_j)
    h'_i = elu( (Σ_j att q_j h_j ...) / denominator )

This version ("single-plane scheme") ships ONE encoded score plane per core,
Y[j,i] = where(A^T, u'_i, -B)  (bf16, B=240), and decomposes
    A*max(u',v) = A*v + A*relu(u'-v):
  - relu part: ONE VectorE tensor_scalar per 128-j chunk (4x mode):
        P2 = max(Y - v_j, 0)            (A=0 cells give relu(-B-v)=0 exactly)
        psN += [q h | q]^T @ P2
  - A*v part is recovered ALGEBRAICALLY from the same plane on the PE:
        psM += [q v h | q v]^T @ Y
        Σ_{j∈A} qvh = (psM + B*C) * 1/(u'_i + B)    (exact; C = Σ_j qvh)
    The B*C offset is folded into psM's PSUM group via a K=1 fp32 matmul at
    start; 1/(u'+B) is a host-computed fp32 row, broadcast via ones-matmul.
Epilogue: R = psN + (psM)*rcp_bc; h' = R[:64]/R[64]; elu; store.

vs the previous kernel this cuts VectorE O(N^2) work from ~43us to ~21us
(one 4x-mode tensor_scalar per chunk instead of TT+TS mixes) and ScalarE
O(N^2) work to zero; PE carries two bf16 passes over the plane (~55us).
(fp8 DoubleRow would halve the P2 pass, but its 64-wide stationary limit
cannot carry the 65th (denominator) functional; walrus also crashes on
M>64 DoubleRow. Measured tradeoff is a wash, so everything stays bf16.)

Sharding: row-parallel over the 8192 output rows, 1024 rows per core,
no cross-core communication.
"""

import numpy as np
import ml_dtypes

N = 8192
IN_F = 128
OUT_F = 64
HT_F = OUT_F + 1  # [q*h | q] carries the denominator column
N_CORES = 8
SLAB = N // N_CORES  # 1024 output rows per core
P = 128
NT = N // P  # 64 j-chunks of 128
NPAIR = NT // 2
HALF = SLAB // 2  # PSUM free-dim limit for fp32 is 512
BIG = 240.0

_bf16 = ml_dtypes.bfloat16
_f8 = ml_dtypes.float8_e4m3  # TRN FP8_EXP4 (max +-240)

_nc_cache = None


def _build_bass():
    import concourse.mybir as mybir
    import concourse.tile as tile
    from concourse import bacc

    bf = mybir.dt.bfloat16
    f8 = mybir.dt.float8e4
    f32 = mybir.dt.float32
    Alu = mybir.AluOpType
    DR = mybir.MatmulPerfMode.DoubleRow

    nc = bacc.Bacc("TRN2", target_bir_lowering=False, debug=False)

    plane = nc.dram_tensor("plane", [N, SLAB], bf, kind="ExternalInput")
    vT = nc.dram_tensor("vT", [P, NT], f32, kind="ExternalInput")
    qh8 = nc.dram_tensor("qh8", [P, NT * HT_F], bf, kind="ExternalInput")
    qvh = nc.dram_tensor("qvh", [P, NT * HT_F], bf, kind="ExternalInput")
    rcp = nc.dram_tensor("rcp", [1, SLAB], f32, kind="ExternalInput")
    bcT = nc.dram_tensor("bcT", [1, HT_F], f32, kind="ExternalInput")
    out = nc.dram_tensor("out", [OUT_F, SLAB], f32, kind="ExternalOutput")

    plane_t = plane.rearrange("(t p) i -> t p i", p=P)

    with tile.TileContext(nc) as tc:
        with (
            tc.tile_pool(name="const", bufs=1) as cpool,
            tc.tile_pool(name="mask", bufs=4) as mpool,
            tc.tile_pool(name="p2", bufs=4) as gpool,
            tc.tile_pool(name="ps", bufs=1, space="PSUM") as pspool,
            tc.tile_pool(name="epi", bufs=1) as epool,
        ):
            vT_sb = cpool.tile([P, NT], f32)
            nc.sync.dma_start(vT_sb[:], vT[:])
            rcp_sb = cpool.tile([1, SLAB], f32)
            nc.scalar.dma_start(rcp_sb[:], rcp[:])
            bc_sb = cpool.tile([1, HT_F], f32)
            nc.scalar.dma_start(bc_sb[:], bcT[:])
            qh8_sb = cpool.tile([P, NT, HT_F], bf)
            nc.scalar.dma_start(qh8_sb[:], qh8.rearrange("p (t f) -> p t f", f=HT_F))
            qvh_sb = cpool.tile([P, NT, HT_F], bf)
            nc.scalar.dma_start(qvh_sb[:], qvh.rearrange("p (t f) -> p t f", f=HT_F))

            ones65 = cpool.tile([1, HT_F], f32)
            nc.vector.memset(ones65[:], 1.0)
            onesrow = cpool.tile([1, HALF], f32)
            nc.vector.memset(onesrow[:], 1.0)

            # Warm the ACT exp table during the main loop (ScalarE is idle).
            warm = cpool.tile([P, 8], f32)
            nc.scalar.activation(
                warm[:], vT_sb[:, 0:8], mybir.ActivationFunctionType.Exp
            )

            # PSUM accumulators (one bank each, 8 banks total)
            psN0 = pspool.tile([HT_F, HALF], f32)
            psN1 = pspool.tile([HT_F, HALF], f32)
            psM0 = pspool.tile([HT_F, HALF], f32)
            psM1 = pspool.tile([HT_F, HALF], f32)
            pb0 = pspool.tile([HT_F, HALF], f32)
            pb1 = pspool.tile([HT_F, HALF], f32)
            pd0 = pspool.tile([OUT_F, HALF], f32)
            pd1 = pspool.tile([OUT_F, HALF], f32)

            # rcp broadcast across the 65 output partitions (off critical path)
            nc.tensor.matmul(pb0[:], ones65[:], rcp_sb[:, 0:HALF])
            nc.tensor.matmul(pb1[:], ones65[:], rcp_sb[:, HALF:SLAB])
            pbS = epool.tile([HT_F, SLAB], f32)
            nc.vector.tensor_copy(out=pbS[:, 0:HALF], in_=pb0[:])
            nc.vector.tensor_copy(out=pbS[:, HALF:SLAB], in_=pb1[:])

            # init psM accumulation with the +B*C offset (K=1 fp32 matmul)
            nc.tensor.matmul(psM0[:], bc_sb[:], onesrow[:], start=True, stop=False)
            nc.tensor.matmul(psM1[:], bc_sb[:], onesrow[:], start=True, stop=False)

            # ---- main loop over j-chunks, grouped for DMA batching ----
            groups = [(0, 1), (1, 1), (2, 2)] + [(4 + 6 * k, 6) for k in range(10)]
            for t0g, grp in groups:
                ybf = mpool.tile([P, 6, SLAB], bf, tag="ybf")
                nc.sync.dma_start(
                    ybf[:, 0:grp, :],
                    plane_t[t0g : t0g + grp].rearrange("t p i -> p t i"),
                )
                p2g = gpool.tile([P, 6, SLAB], bf, tag="p2g")
                for b in range(grp):
                    t = t0g + b
                    # P2 = max(Y - v_j, 0)  -- one 4x-mode tensor_scalar
                    nc.vector.tensor_scalar(
                        p2g[:, b, :],
                        ybf[:, b, :],
                        vT_sb[:, t : t + 1],
                        0.0,
                        Alu.subtract,
                        Alu.max,
                    )
                    # psM += qvh_t^T @ Y_t   (extracts Σ_A qvh via algebra)
                    last = t == NT - 1
                    nc.tensor.matmul(
                        psM0[:], qvh_sb[:, t, :], ybf[:, b, 0:HALF],
                        start=False, stop=last,
                    )
                    nc.tensor.matmul(
                        psM1[:], qvh_sb[:, t, :], ybf[:, b, HALF:SLAB],
                        start=False, stop=last,
                    )
                    # psN += qh_t^T @ P2_t   (relu part + its denominator col)
                    nc.tensor.matmul(
                        psN0[:], qh8_sb[:, t, :], p2g[:, b, 0:HALF],
                        start=(t == 0), stop=last,
                    )
                    nc.tensor.matmul(
                        psN1[:], qh8_sb[:, t, :], p2g[:, b, HALF:SLAB],
                        start=(t == 0), stop=last,
                    )

            # ---- epilogue ----
            # R = psN + psM * rcp_bc   (psM already carries +B*C)
            t1 = epool.tile([HT_F, SLAB], f32)
            R = epool.tile([HT_F, SLAB], f32)
            DEN = slice(OUT_F, HT_F)
            # denominator row first, so its reciprocal round-trip overlaps
            nc.vector.tensor_tensor(
                t1[DEN, 0:HALF], pbS[DEN, 0:HALF], psM0[DEN, :], Alu.mult
            )
            nc.vector.tensor_tensor(
                t1[DEN, HALF:SLAB], pbS[DEN, HALF:SLAB], psM1[DEN, :], Alu.mult
            )
            nc.vector.tensor_tensor(
                R[DEN, 0:HALF], t1[DEN, 0:HALF], psN0[DEN, :], Alu.add
            )
            nc.vector.tensor_tensor(
                R[DEN, HALF:SLAB], t1[DEN, HALF:SLAB], psN1[DEN, :], Alu.add
            )
            den128 = epool.tile([P, SLAB // P], f32)
            nc.sync.dma_start(den128[:], R[OUT_F : OUT_F + 1, :])
            rcp128 = epool.tile([P, SLAB // P], f32)
            nc.vector.reciprocal(out=rcp128[:], in_=den128[:])
            rcpd = epool.tile([1, SLAB], f32)
            nc.sync.dma_start(rcpd[:], rcp128[:])

            FEAT = slice(0, OUT_F)
            nc.vector.tensor_tensor(
                t1[FEAT, 0:HALF], pbS[FEAT, 0:HALF], psM0[FEAT, :], Alu.mult
            )
            nc.vector.tensor_tensor(
                t1[FEAT, HALF:SLAB], pbS[FEAT, HALF:SLAB], psM1[FEAT, :], Alu.mult
            )
            nc.vector.tensor_tensor(
                R[FEAT, 0:HALF], t1[FEAT, 0:HALF], psN0[FEAT, :], Alu.add
            )
            nc.vector.tensor_tensor(
                R[FEAT, HALF:SLAB], t1[FEAT, HALF:SLAB], psN1[FEAT, :], Alu.add
            )

            ones64 = cpool.tile([1, OUT_F], f32)
            nc.vector.memset(ones64[:], 1.0)
            nc.tensor.matmul(pd0[:], ones64[:], rcpd[:, 0:HALF])
            nc.tensor.matmul(pd1[:], ones64[:], rcpd[:, HALF:SLAB])

            div = epool.tile([OUT_F, SLAB], f32)
            nc.vector.tensor_tensor(
                div[:, 0:HALF], R[0:OUT_F, 0:HALF], pd0[:], Alu.mult
            )
            nc.vector.tensor_tensor(
                div[:, HALF:SLAB], R[0:OUT_F, HALF:SLAB], pd1[:], Alu.mult
            )

            # elu(x) = relu(x) + min(exp(x) - 1, 0)  (bf16 intermediates)
            ex = epool.tile([OUT_F, SLAB], bf)
            nc.scalar.activation(ex[:], div[:], mybir.ActivationFunctionType.Exp)
            exm = epool.tile([OUT_F, SLAB], bf)
            nc.vector.tensor_scalar(exm[:], ex[:], 1.0, 0.0, Alu.subtract, Alu.min)
            rl = epool.tile([OUT_F, SLAB], bf)
            nc.vector.tensor_scalar(rl[:], div[:], 0.0, None, Alu.max)
            ov = epool.tile([OUT_F, SLAB], f32)
            nc.vector.tensor_tensor(ov[:], exm[:], rl[:], Alu.add)

            nc.sync.dma_start(out[:], ov[:])

    nc.finalize()
    return nc


def _get_nc():
    global _nc_cache
    if _nc_cache is None:
        _nc_cache = _build_bass()
    return _nc_cache


def prepare_inputs(input, adj, W, a):
    """Host-side O(N*F) precompute + input marshaling (elementwise mask remap
    only). Returns per-core input maps for the SPMD bass kernel."""
    f32 = np.float32
    input = np.asarray(input, dtype=f32)
    W = np.asarray(W, dtype=f32)
    a = np.asarray(a, dtype=f32)
    adj = np.asarray(adj)

    h = input @ W  # [N, 64]
    f_src = h @ a[:OUT_F]
    f_dst = h @ a[OUT_F:]

    u_b = np.exp(-0.8 * f_src).astype(_bf16)  # u' = exp(-0.8 f_src), bf16
    u_f = u_b.astype(f32)
    v = np.exp(0.8 * f_dst).astype(f32)
    q = np.exp(0.2 * f_dst).astype(f32)

    htil = np.empty((N, HT_F), f32)
    htil[:, :OUT_F] = h * q[:, None]
    htil[:, OUT_F] = q
    htil8 = htil.astype(_bf16)
    hv = (htil * v[:, None]).astype(_bf16)

    def dev_layout(x):
        # partition p holds chunk t at columns [t*65, (t+1)*65)
        return np.ascontiguousarray(
            x.reshape(NT, P, HT_F).transpose(1, 0, 2).reshape(P, NT * HT_F)
        )

    qh8_dev = dev_layout(htil8)
    qvh_dev = dev_layout(hv)

    C = hv.astype(np.float64).sum(axis=0)  # consistent with bf16 stationary
    bcT_dev = (BIG * C).astype(f32).reshape(1, HT_F)

    vT_dev = np.ascontiguousarray(v.reshape(NT, P).T)  # [128, 64] f32

    # plane[j, i] = where(adj^T, u'_i, -240) in bf16, built via uint16 bits
    u_bits = u_b.view(np.uint16)
    NEG_BITS = np.uint16(np.float32(-BIG).astype(_bf16).view(np.uint16))
    adjT = np.ascontiguousarray(adj.T != 0)  # [N(j), N(i)]

    in_maps = []
    for c in range(N_CORES):
        sl = slice(c * SLAB, (c + 1) * SLAB)
        plane_c = np.where(adjT[:, sl], u_bits[None, sl], NEG_BITS).view(_bf16)
        rcp_c = (1.0 / (u_f[sl] + BIG)).astype(f32).reshape(1, SLAB)
        in_maps.append(
            {
                "plane": plane_c,
                "vT": vT_dev,
                "qh8": qh8_dev,
                "qvh": qvh_dev,
                "rcp": rcp_c,
                "bcT": bcT_dev,
            }
        )
    return in_maps


def assemble_output(results):
    """results: list of 8 dicts with 'out' [64, 1024] f32 -> [N, 64] f32."""
    hp = np.empty((N, OUT_F), np.float32)
    for c in range(N_CORES):
        hp[c * SLAB : (c + 1) * SLAB] = results[c]["out"].T
    return hp


def kernel(input, adj, W, a):
    import time

    from concourse.bass_utils import run_bass_kernel_spmd

    nc = _get_nc()
    in_maps = prepare_inputs(input, adj, W, a)
    last_err = None
    for attempt in range(3):
        try:
            res = run_bass_kernel_spmd(nc, in_maps, core_ids=list(range(N_CORES)))
            return assemble_output(res.results)
        except Exception as e:  # transient device wedges have been observed
            last_err = e
            time.sleep(5)
    raise last_err
